# revision 77
# baseline (speedup 1.0000x reference)
"""Multi-head attention kernel for Trainium2, 8 NeuronCores.

Problem: B=2, S=4096, D=512, H=8 heads (dk=64), explicit S x S masked softmax.

Sharding (Megatron-style tensor parallel): batch (2) x head-pairs (4) -> 8
cores. Each core computes 2 heads for ALL 4096 queries of one batch element,
with w_q/w_k/w_v column-sliced (128 cols) and w_o row-sliced (128 rows).
Each core emits a PARTIAL [S, D] output (its heads' w_o contribution); the
host sums the 4 partials per batch during unsharding. This removes the 4x
K/V projection duplication of a query-sharded layout: per-core projection
work drops from 180K to 49K PE cycles, so the serial projection prologue
shrinks ~4x and the steady state is ACT(exp)-limited.

Per-core layout choices:
  - scores computed transposed ([keys, q]) so the PV matmul consumes them
    directly (no on-chip transposes anywhere).
  - mask streamed per query-tile as uint8 and DMA-cast to bf16 on load.
  - softmax sums come from a ones-column appended to V (M=65 PV matmul);
    the reciprocal row is broadcast across partitions via a DRAM bounce
    (stride-0 partition DMA); normalize-multiplies are deferred into the
    next unit's pipeline so unit boundaries never stall.
  - both heads' normalized outputs are packed into one [128, QT] tile
    (DVE writes head 1 at partitions 64-127), so the output projection is
    a single 128-contraction matmul per 128 query rows.
  - ALL psum flows through two pools (scores ring 2x3 banks + pv/wo ring
    2x1): projections borrow score-ring slices, so leftover projection
    work (V quarters, Q tiles) drains into early attention groups where
    the PE has slack against the ACT exp rate.
  - all matmul operands bf16 (host- or DMA-cast), fp32 accumulation in PSUM.
"""

import numpy as np

B, S, D, H = 2, 4096, 512, 8
DK = D // H            # 64
NCORES = 8
HP = 2                 # heads per core
NQT = 8                # query tiles per core
QT = 512               # query tile
KBS = 128              # key block size
KB = S // KBS          # 32 key blocks
G = 3                  # key blocks per ACT exp group (3 PSUM banks, FD=1536)
QTR = 1024             # input staging quarter (columns)

_BUILT = None


def _build():
    import concourse.bacc as bacc
    import concourse.mybir as mybir
    import concourse.tile as tile
    from concourse.bass_interp import get_hw_module

    F32 = mybir.dt.float32
    BF16 = mybir.dt.bfloat16
    U8 = mybir.dt.uint8
    EXP = mybir.ActivationFunctionType.Exp
    MULT = mybir.AluOpType.mult

    nc = bacc.Bacc("TRN2", target_bir_lowering=False, debug=False,
                   enable_asserts=False, num_devices=NCORES)

    qT = nc.dram_tensor("qT", [D, S], BF16, kind="ExternalInput")
    kT = nc.dram_tensor("kT", [D, S], BF16, kind="ExternalInput")
    vT = nc.dram_tensor("vT", [D, S], BF16, kind="ExternalInput")
    maskP = nc.dram_tensor("maskP", [NQT, 128, KB, QT], U8,
                           kind="ExternalInput")
    wq = nc.dram_tensor("wq", [D, HP * DK], BF16, kind="ExternalInput")
    wk = nc.dram_tensor("wk", [D, HP * DK], BF16, kind="ExternalInput")
    wv = nc.dram_tensor("wv", [D, HP * DK], BF16, kind="ExternalInput")
    wo = nc.dram_tensor("wo", [HP * DK, D], BF16, kind="ExternalInput")
    out = nc.dram_tensor("out", [S, D], BF16, kind="ExternalOutput")
    # DRAM bounce buffer for broadcasting softmax reciprocals across partitions
    rcd = nc.dram_tensor("rcd", [NQT * HP, 512], F32, kind="Internal")

    import concourse.bass as bass

    with tile.TileContext(nc) as tc:
        with tc.tile_pool(name="persist", bufs=1) as persist, \
             tc.tile_pool(name="maskp", bufs=2) as maskp, \
             tc.tile_pool(name="masku", bufs=2) as masku, \
             tc.tile_pool(name="pstg", bufs=3) as pstg, \
             tc.tile_pool(name="pxt", bufs=2) as pxt, \
             tc.tile_pool(name="pwrk", bufs=3) as pwrk, \
             tc.tile_pool(name="pex", bufs=6) as pex, \
             tc.tile_pool(name="late", bufs=1) as late, \
             tc.tile_pool(name="psc", bufs=2, space="PSUM") as psc, \
             tc.tile_pool(name="ppv", bufs=2, space="PSUM") as ppv:

            ppv._bctag = "pvb"

            # ---------------- persistent tiles ----------------
            KT = persist.tile([128, S], BF16)        # K^T (2 heads' dk rows)
            QTt = persist.tile([128, S], BF16)       # Q^T
            VA = persist.tile([128, KB, HP * 65], BF16)  # V + ones col/head
            ones_t = persist.tile([128, 64], F32)
            nc.vector.memset(ones_t, 1.0)
            WO128 = persist.tile([128, D], BF16)     # both heads' wo rows
            wk_b = persist.tile([128, 4, 128], BF16)
            wq_b = persist.tile([128, 4, 128], BF16)
            wv_b = persist.tile([128, 4, 128], BF16)

            va_ones = VA.rearrange("p kb (h x) -> p kb h x", x=65)[:, :, :, 64:65]
            nc.gpsimd.memset(va_ones, 1.0)

            kT_src = kT[:, :].rearrange("(c p) s -> p c s", p=128)
            vT_src = vT[:, :].rearrange("(c p) s -> p c s", p=128)
            qT_src = qT[:, :].rearrange("(c p) s -> p c s", p=128)

            maskq = {}
            masku_t = {}

            def get_mq(qt):
                if qt not in maskq:
                    maskq[qt] = maskp.tile([128, KB, QT], BF16, tag="mq",
                                           name=f"mq{qt}")
                return maskq[qt]

            def load_masku(qt, quarter):
                # raw uint8 quarter (half the DMA bytes of the casting path)
                if qt not in masku_t:
                    masku_t[qt] = masku.tile([128, KB, QT], U8, tag="mu",
                                             name=f"mu{qt}")
                nc.gpsimd.dma_start(
                    out=masku_t[qt][:, quarter * 8:(quarter + 1) * 8, :],
                    in_=maskP[qt, :, quarter * 8:(quarter + 1) * 8, :])

            def conv_mask(qt, quarter):
                # u8 -> bf16 on the otherwise-idle GPSIMD engine
                nc.gpsimd.tensor_copy(
                    get_mq(qt)[:, quarter * 8:(quarter + 1) * 8, :],
                    masku_t[qt][:, quarter * 8:(quarter + 1) * 8, :])

            def load_mask(qt, mc0=0, mc1=4, coarse=False):
                # fine: 4 preemptible sub-DMAs per quarter so latency-
                # sensitive transfers (rcd bounce) interleave between them.
                # coarse: one DMA per quarter -- saves ~1.1us of serialized
                # Pool desc-gen per chunk; safe before any rcd traffic.
                mq = get_mq(qt)
                if coarse:
                    for q in range(mc0, mc1):
                        nc.gpsimd.dma_start(
                            out=mq[:, q * 8:(q + 1) * 8, :],
                            in_=maskP[qt, :, q * 8:(q + 1) * 8, :])
                    return
                for mc in range(4 * mc0, 4 * mc1):
                    nc.gpsimd.dma_start(
                        out=mq[:, mc * 2:(mc + 1) * 2, :],
                        in_=maskP[qt, :, mc * 2:(mc + 1) * 2, :])

            # ---------------- projections (quarter-staged) ----------------
            # All proj psum borrows score-ring tiles ([128, 3, 512] = 3
            # banks), so projections and attention share one PSUM layout.
            def stage(src, qtr, name):
                t = pstg.tile([128, 4, QTR], BF16, tag="stg", name=name)
                nc.gpsimd.dma_start(
                    out=t, in_=src[:, :, qtr * QTR:(qtr + 1) * QTR])
                return t

            def stage2(src, qtr, name):
                # two half sub-DMAs into one tile: the first projection
                # tile depends only on the first 512 columns, halving the
                # startup-critical DMA latency
                t = pstg.tile([128, 4, QTR], BF16, tag="stg", name=name)
                for hf in range(2):
                    nc.gpsimd.dma_start(
                        out=t[:, :, hf * 512:(hf + 1) * 512],
                        in_=src[:, :, qtr * QTR + hf * 512:
                                qtr * QTR + (hf + 1) * 512])
                return t

            def proj_kq(w_b, tin, dst, toff, nst, name, split_copy=False):
                # dst[:, toff : toff+nst*512] from one staged quarter
                pt = psc.tile([128, G, QT], F32, tag="sc", name=f"pp{name}")
                for ns in range(nst):
                    for di in range(4):
                        nc.tensor.matmul(
                            pt[:, ns, :], w_b[:, di, :],
                            tin[:, di, ns * 512:(ns + 1) * 512],
                            start=(di == 0), stop=(di == 3))
                    if split_copy:
                        # per-tile copies so early consumers wait only on
                        # the columns they actually read
                        nc.vector.tensor_copy(
                            dst[:, toff + ns * 512:toff + (ns + 1) * 512],
                            pt[:, ns, :])
                if not split_copy:
                    nc.vector.tensor_copy(
                        dst[:, toff:toff + nst * 512],
                        pt[:, 0:nst, :].rearrange("p a b -> p (a b)"))

            def proj_v(tin, sc0):
                # 8 VA chunks [128 s-rows, 128 v-cols] from one staged quarter
                pt = psc.tile([128, G, QT], F32, tag="sc", name=f"ppv{sc0}")
                for i in range(8):
                    si = (sc0 + i) % 8
                    for di in range(4):
                        nc.tensor.matmul(
                            pt[:, i // 4, (i % 4) * 128:(i % 4 + 1) * 128],
                            tin[:, di, si * 128:(si + 1) * 128],
                            wv_b[:, di, :],
                            start=(di == 0), stop=(di == 3))
                src = pt.rearrange("p g (qc hx) -> p (g qc) hx", hx=128)
                src = src.rearrange("p c (h x) -> p c h x", x=64)[:, 0:8]
                dst = VA[:, sc0:sc0 + 8, :].rearrange(
                    "p kb (h x) -> p kb h x", x=65)[:, :, :, 0:64]
                nc.vector.tensor_copy(dst, src)

            # prologue DMA order is latency-critical: first-needed first.
            # Only K quarter 0 / Q quarter 0 / mask quarter 0 / V quarter 0
            # are projected before attention emission starts; everything
            # else drains into early attention groups in consumption order.
            def stage_sync(src, qtr, name):
                t = pstg.tile([128, 4, QTR], BF16, tag="stg", name=name)
                nc.sync.dma_start(
                    out=t, in_=src[:, :, qtr * QTR:(qtr + 1) * QTR])
                return t

            def stage_sync(src, qtr, name):
                t = pstg.tile([128, 4, QTR], BF16, tag="stg", name=name)
                nc.sync.dma_start(
                    out=t, in_=src[:, :, qtr * QTR:(qtr + 1) * QTR])
                return t

            kq0 = stage_sync(kT_src, 0, "kq0")
            nc.sync.dma_start(
                out=wk_b, in_=wk[:, :].rearrange("(c p) d -> p c d", p=128))
            qq0 = stage(qT_src, 0, "qq0")
            nc.gpsimd.dma_start(
                out=wq_b, in_=wq[:, :].rearrange("(c p) d -> p c d", p=128))
            load_mask(0, 0, 1)
            vq0 = stage(vT_src, 0, "vq0")
            nc.gpsimd.dma_start(
                out=wv_b, in_=wv[:, :].rearrange("(c p) d -> p c d", p=128))
            proj_kq(wk_b, kq0, KT, 0, 2, "k0", split_copy=True)
            kq1 = stage(kT_src, 1, "kq1")
            proj_kq(wq_b, qq0, QTt, 0, 2, "q0", split_copy=True)
            load_mask(0, 1, 2)
            vq1 = stage(vT_src, 1, "vq1")
            staged = {("kq", 1): kq1, ("v", 0): vq0, ("v", 1): vq1}

            # drained actions, keyed by (unit, group) emission slots; each
            # runs on the shared psc ring / DMA queues in consumption order
            def a_dma_kq(qtr):
                return lambda: staged.__setitem__(
                    ("kq", qtr), stage(kT_src, qtr, f"kq{qtr}"))

            def a_dma_vq(qtr):
                return lambda: staged.__setitem__(
                    ("v", qtr), stage(vT_src, qtr, f"vq{qtr}"))

            def a_dma_qq(qtr):
                return lambda: staged.__setitem__(
                    ("q", qtr), stage(qT_src, qtr, f"qq{qtr}"))

            def a_k(qtr):
                return lambda: proj_kq(wk_b, staged.pop(("kq", qtr)), KT,
                                       qtr * QTR, 2, f"k{qtr}")

            def a_v(qtr):
                return lambda: proj_v(staged.pop(("v", qtr)), qtr * 8)

            def a_q(qtr):
                return lambda: proj_kq(wq_b, staged.pop(("q", qtr)), QTt,
                                       qtr * QTR, 2, f"q{qtr}")

            def a_m0(quarter):
                return lambda: load_mask(0, quarter, quarter + 1)

            def a_m1(quarter):
                return lambda: load_mask(1, quarter, quarter + 1)

            def a_wo():
                return lambda: nc.gpsimd.dma_start(out=WO128, in_=wo[:, :])

            drain_slots = {
                (0, 0): [a_k(1), a_v(0), a_m0(2)],
                (0, 1): [a_dma_kq(2), a_dma_vq(2)],
                (0, 2): [a_v(1), a_m0(3)],
                (0, 3): [a_k(2)],
                (0, 5): [a_v(2), a_dma_kq(3), a_dma_vq(3)],
                (0, 6): [a_k(3)],
                (0, 8): [a_v(3)],
                (0, 10): [a_wo()],
                (2, 1): [a_dma_qq(1)],
                (2, 5): [a_q(1)],
                (2, 7): [a_dma_qq(2)],
                (3, 5): [a_q(2)],
                (3, 7): [a_dma_qq(3)],
                (4, 5): [a_q(3)],
            }

            def drain_task(ui, gi):
                for a in drain_slots.get((ui, gi), ()):
                    a()

            # ---------------- attention ----------------
            units = [(qt, h) for qt in range(NQT) for h in range(HP)]
            xts = {}

            def emit_norm1(ui, pv):
                # reciprocal of the sums row, bounced through DRAM to
                # broadcast it across partitions 0-63 (SBUF APs cannot
                # have a zero partition stride; DRAM APs can). The SP
                # queue carries only this + the small out-writes, so the
                # latency-sensitive bounce never queues behind bulk DMAs.
                rc = pwrk.tile([128, QT], F32, tag="rc", name=f"rc{ui}")
                nc.vector.reciprocal(rc[64:65, :], pv[64:65, :])
                nc.sync.dma_start(out=rcd[ui:ui + 1, :], in_=rc[64:65, :])
                bcs = pwrk.tile([64, QT], F32, tag="bcs", name=f"bcs{ui}")
                src = rcd[ui:ui + 1, :]
                bsrc = bass.AP(tensor=src.tensor, offset=src.offset,
                               ap=[[0, 64]] + [list(a) for a in src.ap[1:]])
                nc.sync.dma_start(out=bcs, in_=bsrc)
                return bcs

            def emit_norm2(qt, h, pv, bcs):
                # head h lands at partitions h*64 .. h*64+63 (packed xt)
                nc.vector.tensor_tensor(
                    xts[qt][h * 64:(h + 1) * 64, :], pv[0:64, :], bcs, op=MULT)

            groups = []
            kb0 = 0
            while kb0 < KB:
                groups.append((kb0, min(G, KB - kb0)))
                kb0 += G

            def emit_scores(qt, h, kb0, gn):
                pb = h * 64
                sc = psc.tile([128, G, QT], F32, tag="sc")
                for i in range(gn):
                    kb = kb0 + i
                    nc.tensor.matmul(
                        sc[:, i, :],
                        KT[pb:pb + 64, kb * 128:(kb + 1) * 128],
                        QTt[pb:pb + 64, qt * QT:(qt + 1) * QT],
                        start=True, stop=True)
                return sc

            pending_wo = []

            def emit_wo(qt, qc, use_act=False):
                po = ppv.tile([128, D], F32, tag="pvb", name=f"po{qt}_{qc}")
                nc.tensor.matmul(
                    po, xts[qt][:, qc * 128:(qc + 1) * 128], WO128,
                    start=True, stop=True)
                outt = late.tile([128, D], BF16, tag="outt", bufs=4)
                if use_act:
                    # final flush only: ACT is idle once the last exp is done
                    nc.scalar.copy(outt, po)
                else:
                    nc.vector.tensor_copy(outt, po)
                nc.sync.dma_start(
                    out=out[qt * QT + qc * 128:qt * QT + (qc + 1) * 128, :],
                    in_=outt)

            flat = []
            for ui in range(len(units)):
                qt, h = units[ui]
                for gi, (kb0, gn) in enumerate(groups):
                    flat.append((ui, qt, h, gi, kb0, gn))
            # interleave the two units of qt0 and qt1 group-by-group: the
            # early window is input-DMA/convert-bound, and both units of a
            # pair consume the same K/V/mask chunks -- alternating them
            # doubles the exp work available per arrived chunk. WO pops are
            # suppressed until unit 4 (interleaved pairs hold both pv psum
            # slots, so a po allocation would stall the PE queue).
            ng = len(groups)

            def pair_interleave(fl, pair):
                base = pair * 2 * ng
                head = fl[base:base + 2 * ng]
                inter = []
                for g in range(ng):
                    inter.append(head[g])
                    inter.append(head[ng + g])
                return fl[:base] + inter + fl[base + 2 * ng:]

            flat = pair_interleave(flat, 0)

            sc_tiles = {}

            def emit_sc(idx):
                _, qt, h, _, kb0, gn = flat[idx]
                sc_tiles[idx] = emit_scores(qt, h, kb0, gn)

            emit_sc(0)
            emit_sc(1)
            pv_of = {}
            pending_norms = []
            for idx, (ui, qt, h, gi, kb0, gn) in enumerate(flat):
                if h == 0 and gi == 0 and qt not in xts:
                    xts[qt] = pxt.tile([128, QT], BF16, tag="xt",
                                       name=f"xt{qt}")
                if h == 0 and gi in (0, 3, 6, 9) and qt + 1 < NQT:
                    load_masku(qt + 1, gi // 3)
                if h == 1 and gi in (1, 4, 7, 10) and qt + 1 < NQT:
                    conv_mask(qt + 1, {1: 0, 4: 1, 7: 2, 10: 3}[gi])
                if gi == 0:
                    pv_of[ui] = ppv.tile([128, QT], F32, tag="pvb",
                                         name=f"pv{ui}")
                pv = pv_of[ui]
                # drains BEFORE the score prefetch: drained projections must
                # precede, in emission order, any consumer of their outputs
                drain_task(ui, gi)
                if idx + 2 < len(flat):
                    emit_sc(idx + 2)
                sc = sc_tiles.pop(idx)
                ex = pex.tile([128, G, QT], BF16, tag="ex")
                nc.scalar.activation(ex[:, 0:gn, :], sc[:, 0:gn, :],
                                     EXP, scale=0.125)
                mk = pex.tile([128, G, QT], BF16, tag="mk")
                nc.vector.tensor_tensor(
                    mk[:, 0:gn, :], ex[:, 0:gn, :],
                    maskq[qt][:, kb0:kb0 + gn, :], op=MULT)
                for i in range(gn):
                    kb = kb0 + i
                    nc.tensor.matmul(
                        pv[0:65, :],
                        VA[:, kb, h * 65:(h + 1) * 65],
                        mk[:, i, :],
                        start=(kb == 0), stop=(kb == KB - 1))
                if gi in (2, 4) and pending_norms:
                    # deferred normalize-multiply of a previous unit (its
                    # pv slot frees here, mid-unit, so the boundary never
                    # serializes on the norm chain)
                    emit_norm2(*pending_norms.pop(0))
                elif gi in (4, 6, 8, 9) and pending_wo:
                    emit_wo(*pending_wo.pop(0))
                if gi == len(groups) - 1:
                    if ui == len(units) - 1:
                        # final unit: fast-path norm via PE broadcast
                        # (score PSUM slots are free at this point)
                        rc = pwrk.tile([128, QT], F32, tag="rc", name="rcF")
                        nc.vector.reciprocal(rc[64:65, :], pv[64:65, :])
                        bct = psc.tile([128, G, QT], F32, tag="sc",
                                       name="bcF")
                        nc.tensor.matmul(bct[0:64, 0, :], ones_t[64:65, :],
                                         rc[64:65, :], start=True, stop=True)
                        bcs = pwrk.tile([64, QT], F32, tag="bcs", name="bcsF")
                        nc.vector.tensor_copy(bcs, bct[0:64, 0, :])
                        emit_norm2(qt, h, pv, bcs)
                        pending_wo.extend(
                            (qt, qc) for qc in range(QT // 128))
                        while pending_wo:
                            emit_wo(*pending_wo.pop(0))
                    else:
                        bcs = emit_norm1(ui, pv)
                        pending_norms.append((qt, h, pv, bcs))
                        if h == HP - 1:
                            pending_wo.extend(
                                (qt, qc) for qc in range(QT // 128))

    nc.compile()
    nc.m = get_hw_module(nc.m)
    return nc


def _get_built():
    global _BUILT
    if _BUILT is None:
        _BUILT = _build()
    return _BUILT


def kernel(q, k, v, mask, w_q, w_k, w_v, w_o):
    import os
    # NTFF tracing needs antenv.axon_hooks, absent in some environments;
    # never let an inherited BASS_TRACE env var route us into that path.
    os.environ.setdefault("BASS_NEVER_TRACE", "1")
    import ml_dtypes
    from concourse.bass_utils import run_bass_kernel_spmd

    bf16 = ml_dtypes.bfloat16

    q = np.asarray(q, dtype=np.float32)
    k = np.asarray(k, dtype=np.float32)
    v = np.asarray(v, dtype=np.float32)
    mask = np.asarray(mask, dtype=np.int32)
    w_q = np.asarray(w_q, dtype=np.float32)
    w_k = np.asarray(w_k, dtype=np.float32)
    w_v = np.asarray(w_v, dtype=np.float32)
    w_o = np.asarray(w_o, dtype=np.float32)

    nc = _get_built()

    qT = [np.ascontiguousarray(q[b].T).astype(bf16) for b in range(B)]
    kT = [np.ascontiguousarray(k[b].T).astype(bf16) for b in range(B)]
    vT = [np.ascontiguousarray(v[b].T).astype(bf16) for b in range(B)]
    # maskP[qt, p, kb, q] = mask[b, qt*512+q, kb*128+p], pre-arranged so
    # each query tile's mask is one contiguous-run DMA
    maskP = [np.ascontiguousarray(
        mask[b].astype(np.uint8).reshape(NQT, QT, KB, 128)
        .transpose(0, 3, 2, 1)) for b in range(B)]

    in_maps = []
    for c in range(NCORES):
        b, hp = divmod(c, 4)
        cs = hp * HP * DK
        ce = cs + HP * DK
        in_maps.append({
            "qT": qT[b], "kT": kT[b], "vT": vT[b], "maskP": maskP[b],
            "wq": np.ascontiguousarray(w_q[:, cs:ce]).astype(bf16),
            "wk": np.ascontiguousarray(w_k[:, cs:ce]).astype(bf16),
            "wv": np.ascontiguousarray(w_v[:, cs:ce]).astype(bf16),
            "wo": np.ascontiguousarray(w_o[cs:ce, :]).astype(bf16),
        })

    global _LAST_IN_MAPS
    _LAST_IN_MAPS = in_maps
    res = run_bass_kernel_spmd(nc, in_maps, list(range(NCORES)))

    # Megatron row-parallel unshard: sum the 4 partial w_o contributions
    full = np.empty((B, S, D), dtype=np.float32)
    for b in range(B):
        acc = np.zeros((S, D), dtype=np.float32)
        for hp in range(4):
            acc += np.asarray(res.results[b * 4 + hp]["out"],
                              dtype=np.float32)
        full[b] = acc
    return full
